# revision 1
# baseline (speedup 1.0000x reference)
"""DecoderTreeLSTMCell Trainium2 Bass kernel.

Strategy: data-parallel over nodes on 8 cores (4096 nodes/core). On the host,
each core's nodes are grouped by `pos` (10 groups) and within each group
ordered [mask=0 | mask=1], each side sub-ordered [depth!=1,2 | d==1 | d==2],
with padded compile-time capacities. fp32r (full fp32 bits, 4x PE streaming
rate) is used for the matmul operands.

All per-node inputs are packed into ONE feature-major tensor AIN [128, Lin]
with per-chunk blocks [child_h(C) | child_c(C) | extras(E)], and outputs into
ONE tensor OUT [128, Lout] with blocks [h_new(M0) | c_new(M0) | c_red(C-M0)].
Chunks are loaded/stored in multi-chunk slabs (one DMA each) because each
dma_start costs ~0.6us of serialized HWDGE time.

Per chunk the device computes: h_cat = child_h (+ extras on the depth
sub-ranges, no masks needed), u = W_f[pos].T @ h_cat over all C columns,
f = sigmoid(u + b_f[pos]), c_red = f * child_c. For the mask=0 columns only
it also computes the i/o/uu matmuls, gates, and c_new/h_new. c is stored as
[c_new | c_red] directly (no blend ops). h rows with mask=1 equal h_prev
exactly and are filled host-side during unshard (data routing only - all
arithmetic happens on device).

The reference computes all 10 pos-matmuls for every node and selects; this
kernel computes only the matmul each node needs, placing it near the DMA
roofline.
"""
import numpy as np

import concourse.bacc as bacc
import concourse.mybir as mybir
from concourse.tile import TileContext
from concourse.bass_utils import run_bass_kernel_spmd

N = 32768
H = 128
N_POS = 10
NC = 8
SH = N // NC  # nodes per core

F32 = mybir.dt.float32
F32R = mybir.dt.float32r
Sig = mybir.ActivationFunctionType.Sigmoid
Tanh = mybir.ActivationFunctionType.Tanh

SLAB_CHUNKS = 3  # chunks per DMA slab

# module-level stash for test harness introspection
LAST = {}


def _roundup(x, m):
    return ((x + m - 1) // m) * m


def _plan(pos, depth, mask):
    """Compute per-core slot layout and DMA packing.

    Returns (chunks, slabs, L, Lin, Lout, slot_idx, ain_slot, ain_kind,
    out_slot, out_kind).

    chunks: (p, off, C, M0, e_ranges, ain_off, out_off) - e_ranges are
    (lo, hi) chunk-relative h-column ranges needing the extras add; the
    packed extras for them sit at ain cols [ain_off+2C ...] sequentially.
    slabs: (ain_off, ain_len, out_off, out_len, [chunk indices]).
    slot_idx: [NC, L] original node index per slot (-1 = pad).
    ain_slot/ain_kind: [Lin] mapping of AIN columns to (slot, kind)
    with kind 0=child_h 1=child_c 2=extras. out_slot/out_kind: [Lout]
    mapping of OUT columns, kind 0=h_new 1=c_new 2=c_red.
    """
    dcl = np.where(depth == 1, 1, np.where(depth == 2, 2, 0))
    idx = {}
    counts = np.zeros((NC, N_POS, 2, 3), np.int64)
    # deal each (pos, mask, dclass) bucket round-robin across cores so
    # per-core counts are equal +-1 -> capacities (max over cores) carry
    # almost no padding
    for p in range(N_POS):
        for m in range(2):
            for k in range(3):
                gg = np.nonzero((pos == p) & (mask == m) & (dcl == k))[0]
                for c in range(NC):
                    ii = gg[c::NC]
                    idx[(c, p, m, k)] = ii
                    counts[c, p, m, k] = len(ii)

    caps = np.zeros((N_POS, 2, 3), np.int64)
    for p in range(N_POS):
        for m in range(2):
            for k in range(3):
                caps[p, m, k] = _roundup(int(counts[:, p, m, k].max()), 4)

    def emit(p, off, span_lo, span_hi, m0_hi, espans, out):
        # split [span_lo, span_hi) into <=512 pieces; m0_hi marks the end of
        # the full-pipeline (mask=0) region in pos-block coordinates
        start = span_lo
        while start < span_hi:
            end = min(start + 512, span_hi)
            C = end - start
            M0 = min(max(m0_hi - start, 0), C)
            e = []
            for (lo, hi) in espans:
                l2, h2 = max(lo, start), min(hi, end)
                if l2 < h2:
                    e.append((l2 - start, h2 - start))
            out.append((p, off + start, C, M0, e))
            start = end

    raw_chunks = []  # (p, off, C, M0, e_ranges)
    sub_off = np.zeros((N_POS, 2, 3), np.int64)
    off = 0
    for p in range(N_POS):
        m0n = int(caps[p, 0].sum())
        m1n = int(caps[p, 1].sum())
        M0 = m0n
        w0 = int(caps[p, 0, 1] + caps[p, 0, 2])
        w1 = int(caps[p, 1, 1] + caps[p, 1, 2])
        for k in range(3):
            sub_off[p, 0, k] = off + int(caps[p, 0, :k].sum())
            sub_off[p, 1, k] = off + M0 + int(caps[p, 1, :k].sum())
        espans = []
        if w0:
            espans.append((m0n - w0, m0n))
        if w1:
            espans.append((M0 + m1n - w1, M0 + m1n))
        # always split at the mask0/mask1 boundary for finer pipelining;
        # emit() further subdivides if a side exceeds 512
        emit(p, off, 0, M0, M0, espans, raw_chunks)
        if m1n:
            emit(p, off, M0, M0 + m1n, M0, espans, raw_chunks)
        off += M0 + m1n
    L = off

    slot_idx = np.full((NC, L), -1, np.int64)
    for c in range(NC):
        for p in range(N_POS):
            for m in range(2):
                for k in range(3):
                    ii = idx[(c, p, m, k)]
                    o = int(sub_off[p, m, k])
                    slot_idx[c, o:o + len(ii)] = ii

    # packing: AIN blocks [h(C) | c(C) | e(E)], OUT blocks [h_new | c_new | cr]
    chunks = []
    ain_slot, ain_kind, out_slot, out_kind = [], [], [], []
    a = 0
    o = 0
    for (p, off, C, M0, e_ranges) in raw_chunks:
        chunks.append((p, off, C, M0, e_ranges, a, o))
        ain_slot.extend(range(off, off + C)); ain_kind.extend([0] * C)
        ain_slot.extend(range(off, off + C)); ain_kind.extend([1] * C)
        for (lo, hi) in e_ranges:
            ain_slot.extend(range(off + lo, off + hi))
            ain_kind.extend([2] * (hi - lo))
        a += 2 * C + sum(hi - lo for lo, hi in e_ranges)
        if M0 > 0:
            out_slot.extend(range(off, off + M0)); out_kind.extend([0] * M0)
            out_slot.extend(range(off, off + M0)); out_kind.extend([1] * M0)
        if C > M0:
            out_slot.extend(range(off + M0, off + C))
            out_kind.extend([2] * (C - M0))
        o += M0 + C
    Lin, Lout = a, o

    slabs = []
    for s in range(0, len(chunks), SLAB_CHUNKS):
        grp = list(range(s, min(s + SLAB_CHUNKS, len(chunks))))
        a0 = chunks[grp[0]][5]
        o0 = chunks[grp[0]][6]
        last = chunks[grp[-1]]
        a1 = last[5] + 2 * last[2] + sum(hi - lo for lo, hi in last[4])
        o1 = last[6] + last[3] + last[2]
        slabs.append((a0, a1 - a0, o0, o1 - o0, grp))

    return (chunks, slabs, L, Lin, Lout, slot_idx,
            np.array(ain_slot), np.array(ain_kind),
            np.array(out_slot), np.array(out_kind))


def _build(plan, reps=1, bodies=1):
    chunks, slabs, L, Lin, Lout = plan[:5]
    nc = bacc.Bacc("TRN2", target_bir_lowering=False)
    AIN = nc.dram_tensor("AIN", [H, Lin], F32R, kind="ExternalInput")
    W = nc.dram_tensor("W", [H, N_POS * 4 * H], F32R, kind="ExternalInput")
    BIAS = nc.dram_tensor("BIAS", [H, 13], F32, kind="ExternalInput")
    OUT = nc.dram_tensor("OUT", [H, Lout], F32, kind="ExternalOutput")

    with TileContext(nc) as tc:
        with (
            tc.tile_pool(name="const", bufs=1) as cpool,
            tc.tile_pool(name="io", bufs=5) as io,
            tc.tile_pool(name="wk", bufs=4) as wk,
            tc.tile_pool(name="ps_u", bufs=2, space="PSUM") as ps_u,
            tc.tile_pool(name="ps_i", bufs=2, space="PSUM") as ps_i,
            tc.tile_pool(name="ps_o", bufs=2, space="PSUM") as ps_o,
            tc.tile_pool(name="ps_t", bufs=2, space="PSUM") as ps_t,
        ):
            bias_sb = cpool.tile([H, 13], F32, tag="bias")
            nc.sync.dma_start(out=bias_sb[:, :], in_=BIAS[:, :])
            w_tiles = {}

            def w_load(p):
                if p not in w_tiles:
                    t = cpool.tile([H, 4 * H], F32R, tag=f"w{p}")
                    nc.sync.dma_start(
                        out=t[:, :], in_=W[:, p * 4 * H:(p + 1) * 4 * H])
                    w_tiles[p] = t
                return w_tiles[p]

            def body(_iv=None):
                for (a0, alen, o0, olen, grp) in slabs:
                    ain = io.tile([H, alen], F32R, tag="ain")
                    nc.sync.dma_start(out=ain[:, :], in_=AIN[:, a0:a0 + alen])
                    out = io.tile([H, olen], F32, tag="out")

                    # extras adds first (keeps slab-tile write/read ordering
                    # simple for the scheduler)
                    for ci in grp:
                        (p, off, C, M0, e_ranges, ca, co) = chunks[ci]
                        ra = ca - a0
                        eoff = ra + 2 * C
                        for (lo, hi) in e_ranges:
                            w_ = hi - lo
                            nc.vector.tensor_add(
                                ain[:, ra + lo:ra + hi],
                                ain[:, ra + lo:ra + hi],
                                ain[:, eoff:eoff + w_])
                            eoff += w_

                    for ci in grp:
                        (p, off, C, M0, e_ranges, ca, co) = chunks[ci]
                        ra = ca - a0
                        ro = co - o0
                        h_v = ain[:, ra:ra + C]
                        c_v = ain[:, ra + C:ra + 2 * C].bitcast(F32)
                        w_sb = w_load(p)
                        wof = 0

                        p_u = ps_u.tile([H, C], F32, tag="u")
                        nc.tensor.matmul(p_u[:, :], w_sb[:, wof:wof + H],
                                         h_v, start=True, stop=True)
                        f_sb = wk.tile([H, C], F32, tag="f")
                        nc.scalar.activation(f_sb[:, :], p_u[:, :], Sig,
                                             bias=bias_sb[:, p:p + 1])

                        if M0 > 0:
                            cr_sb = wk.tile([H, M0], F32, tag="cr")
                            nc.vector.tensor_mul(cr_sb[:, :], f_sb[:, 0:M0],
                                                 c_v[:, 0:M0])
                            if C > M0:
                                nc.vector.tensor_mul(
                                    out[:, ro + 2 * M0:ro + M0 + C],
                                    f_sb[:, M0:C], c_v[:, M0:C])

                            p_i = ps_i.tile([H, M0], F32, tag="i")
                            nc.tensor.matmul(p_i[:, :],
                                             w_sb[:, wof + H:wof + 2 * H],
                                             h_v[:, 0:M0], start=True,
                                             stop=True)
                            p_o = ps_o.tile([H, M0], F32, tag="o")
                            nc.tensor.matmul(p_o[:, :],
                                             w_sb[:, wof + 2 * H:wof + 3 * H],
                                             h_v[:, 0:M0], start=True,
                                             stop=True)
                            p_t = ps_t.tile([H, M0], F32, tag="t")
                            nc.tensor.matmul(p_t[:, :],
                                             w_sb[:, wof + 3 * H:wof + 4 * H],
                                             h_v[:, 0:M0], start=True,
                                             stop=True)

                            si_sb = wk.tile([H, M0], F32, tag="si")
                            nc.scalar.activation(si_sb[:, :], p_i[:, :], Sig,
                                                 bias=bias_sb[:, 10:11])
                            tu_sb = wk.tile([H, M0], F32, tag="tu")
                            nc.scalar.activation(tu_sb[:, :], p_t[:, :], Tanh,
                                                 bias=bias_sb[:, 12:13])
                            nc.vector.tensor_mul(si_sb[:, :], si_sb[:, :],
                                                 tu_sb[:, :])
                            c_new = out[:, ro + M0:ro + 2 * M0]
                            nc.vector.tensor_add(c_new, si_sb[:, :],
                                                 cr_sb[:, :])

                            so_sb = wk.tile([H, M0], F32, tag="so")
                            nc.scalar.activation(so_sb[:, :], p_o[:, :], Sig,
                                                 bias=bias_sb[:, 11:12])
                            th_sb = wk.tile([H, M0], F32, tag="th")
                            nc.scalar.activation(th_sb[:, :], c_new, Tanh)
                            nc.vector.tensor_mul(out[:, ro:ro + M0],
                                                 so_sb[:, :], th_sb[:, :])
                        else:
                            # u-only chunk: c_red straight into OUT block
                            nc.vector.tensor_mul(out[:, ro:ro + C],
                                                 f_sb[:, :], c_v)

                    nc.sync.dma_start(out=OUT[:, o0:o0 + olen], in_=out[:, :])

            if reps == 1:
                body()
            else:
                for p_ in range(N_POS):
                    w_load(p_)
                with tc.For_i(0, reps, 1) as _i:
                    for _ in range(bodies):
                        body(_i)
    nc.finalize()
    return nc


_BUILD_CACHE = {}


def _prepare(inputs, reps=1, bodies=1):
    global N, H, N_POS, SH
    N, _, H = np.asarray(inputs["child_h"]).shape
    N_POS = np.asarray(inputs["W_f"]).shape[0] // H
    SH = N // NC
    child_h = np.asarray(inputs["child_h"], np.float32).reshape(N, H)
    child_c = np.asarray(inputs["child_c"], np.float32).reshape(N, H)
    e1 = np.asarray(inputs["extra_input_depth_1"], np.float32)
    e2 = np.asarray(inputs["extra_input_depth_2"], np.float32)
    h_prev = np.asarray(inputs["h_prev"], np.float32)
    pos = np.asarray(inputs["pos"]).astype(np.int64)
    depth = np.asarray(inputs["depth"]).astype(np.int64)
    mask = np.asarray(inputs["mask"]).astype(np.int64)
    W_f = np.asarray(inputs["W_f"], np.float32)
    b_f = np.asarray(inputs["b_f"], np.float32)
    W_iou = np.asarray(inputs["W_iou"], np.float32)
    b_iou = np.asarray(inputs["b_iou"], np.float32)

    mask01 = (mask != 0).astype(np.int64)
    plan = _plan(pos, depth, mask01)
    (chunks, slabs, L, Lin, Lout, slot_idx,
     ain_slot, ain_kind, out_slot, out_kind) = plan

    key = (tuple((p, o, C, M0, tuple(e), ca, co)
                 for p, o, C, M0, e, ca, co in chunks), Lin, Lout, reps)
    if key not in _BUILD_CACHE:
        _BUILD_CACHE[key] = _build(plan, reps=reps, bodies=bodies)
    nc = _BUILD_CACHE[key]

    # weights packed [H, 10*4*H]: per pos p: [W_f_p | Wi0^T | Wi1^T | Wi2^T]
    Wp = np.empty((H, N_POS * 4 * H), np.float32)
    W_f_r = W_f.reshape(N_POS, H, H)
    for p in range(N_POS):
        base = p * 4 * H
        Wp[:, base:base + H] = W_f_r[p]
        for j in range(3):
            Wp[:, base + (j + 1) * H:base + (j + 2) * H] = \
                W_iou[j * H:(j + 1) * H, p * H:(p + 1) * H].T
    bias = np.empty((H, 13), np.float32)
    bias[:, :N_POS] = b_f.reshape(N_POS, H).T
    bias[:, 10] = b_iou[0, 0:H]
    bias[:, 11] = b_iou[0, H:2 * H]
    bias[:, 12] = b_iou[0, 2 * H:3 * H]

    # e source per node: e1 where depth==1, e2 where depth==2 (others unused)
    e_src = np.where((depth == 1)[:, None], e1, e2).astype(np.float32)
    srcs = (child_h, child_c, e_src)

    in_maps = []
    for c in range(NC):
        node = slot_idx[c][ain_slot]          # [Lin] node per ain col, -1 pad
        AIN = np.zeros((H, Lin), np.float32)
        for kind in range(3):
            m = (ain_kind == kind) & (node >= 0)
            AIN[:, m] = srcs[kind][node[m]].T
        in_maps.append({"AIN": AIN, "W": Wp, "BIAS": bias})

    mask_on = mask != 0

    def assemble(results):
        h = np.empty((N, H), np.float32)
        cc = np.empty((N, H), np.float32)
        for c in range(NC):
            node = slot_idx[c][out_slot]      # [Lout] node per out col
            O = results[c]["OUT"]
            mh = (out_kind == 0) & (node >= 0)
            h[node[mh]] = O[:, mh].T
            mc = (out_kind != 0) & (node >= 0)
            cc[node[mc]] = O[:, mc].T
        h[mask_on] = h_prev[mask_on]
        return h, cc

    return nc, in_maps, assemble


def kernel(**inputs):
    nc, in_maps, assemble = _prepare(inputs)
    try:
        res = run_bass_kernel_spmd(nc, in_maps, list(range(NC)))
    except Exception:
        # first execution of a freshly compiled NEFF occasionally kills the
        # worker (transient); one retry has always succeeded
        res = run_bass_kernel_spmd(nc, in_maps, list(range(NC)))
    LAST["results"] = res
    LAST["nc"] = nc
    return assemble(res.results)



# revision 5
# speedup vs baseline: 1.1006x; 1.1006x over previous
"""DecoderTreeLSTMCell Trainium2 Bass kernel.

Strategy: data-parallel over nodes on 8 cores (4096 nodes/core). Host groups
each core's nodes into a column layout [mask0 region | mask1 region], each
region ordered by pos (10 blocks), each block sub-ordered [d0 | d1 | d2]
(d1/d2 = nodes needing the extras add, kept at the block tail). Capacities are
compile-time (max over cores, round-robin deal => ~zero padding).

Everything streams as fp16: ONE input tensor AIN [128, Lin] fp16 with regions
[child_h(L) | child_c(L) | extras(E)], ONE output OUT [128, Lout] fp16 with
regions [h_new(M0) | c_new(M0) | c_red(M1)] - a single DMA each way per pass
(HWDGE descriptor-gen is ~0.6us of serialized queue time per dma_start).

The per-pos f bias b_f is folded into the u matmul via a K=10 one-hot matmul
(stationary = [10,128] b_f rows, moving = [10,C] pos-indicator, PSUM
accumulate). That makes every activation pos-independent, so sigmoids/tanhs
run as one ACT instruction per full 512-col PSUM window instead of per
(pos,mask) chunk - the ACT engine (1 elem/cycle/lane, the bottleneck at
~12.3K cols/core) spends its time on elements, not instruction overhead.
i/o/u biases ride the ACT bias AP (free). All DVE elementwise work is fp16
(2x mode). Matmuls are fp16 (1 cycle/row vs fp32r's 4x penalty below 256
cols).

Per mask0 window: u/i/o/t matmuls per pos piece + one bias matmul, then
f=sig(u), si=sig(i+bi), tu=tanh(t+bu), c_red=f*c, c_new=si*tu+c_red,
so=sig(o+bo), h=so*tanh(c_new). Mask1 windows: u matmuls + f=sig(u),
c_red=f*c only. h rows with mask=1 equal h_prev exactly and are filled
host-side during unshard (data routing only).
"""
import numpy as np

import concourse.bacc as bacc
import concourse.mybir as mybir
from concourse.tile import TileContext
from concourse.bass_utils import run_bass_kernel_spmd

N = 32768
H = 128
N_POS = 10
NC = 8
SH = N // NC

F32 = mybir.dt.float32
F16 = mybir.dt.float16
Sig = mybir.ActivationFunctionType.Sigmoid
Tanh = mybir.ActivationFunctionType.Tanh

WIN = 512  # PSUM bank = 512 fp32 cols

LAST = {}


def _roundup(x, m):
    return ((x + m - 1) // m) * m


def _plan(pos, depth, mask):
    """Column layout + window/piece schedule.

    Returns dict with:
      L, M0, Lin, Lout, Etot
      blocks: {(p, m): (off, c0, w)}  off absolute in L, c0 = no-extras count,
              w = extras tail width; block len = c0 + w
      e_off: {(p, m): offset of its extras columns within the E region}
      windows: list of (m, lo, hi, pieces) absolute [lo,hi) in L,
               pieces = [(p, plo, phi)] absolute
      slot_idx [NC, L], ain_slot/ain_kind [Lin], out_slot/out_kind [Lout]
    """
    dcl = np.where(depth == 1, 1, np.where(depth == 2, 2, 0))
    idx = {}
    counts = np.zeros((NC, N_POS, 2, 3), np.int64)
    for p in range(N_POS):
        for m in range(2):
            for k in range(3):
                gg = np.nonzero((pos == p) & (mask == m) & (dcl == k))[0]
                for c in range(NC):
                    ii = gg[c::NC]
                    idx[(c, p, m, k)] = ii
                    counts[c, p, m, k] = len(ii)

    caps = np.zeros((N_POS, 2, 3), np.int64)
    for p in range(N_POS):
        for m in range(2):
            for k in range(3):
                caps[p, m, k] = _roundup(int(counts[:, p, m, k].max()), 4)

    blocks = {}
    sub_off = {}
    e_off = {}
    off = 0
    eo = 0
    region_lo = [0, 0]
    for m in range(2):
        region_lo[m] = off
        for p in range(N_POS):
            c0 = int(caps[p, m, 0])
            w = int(caps[p, m, 1] + caps[p, m, 2])
            blocks[(p, m)] = (off, c0, w)
            for k in range(3):
                sub_off[(p, m, k)] = off + int(caps[p, m, :k].sum())
            e_off[(p, m)] = eo
            eo += w
            off += c0 + w
    M0 = region_lo[1]
    L = off
    Etot = eo
    Lin = 2 * L + Etot
    Lout = 2 * M0 + (L - M0)

    windows = []
    for m in range(2):
        r0 = region_lo[m]
        r1 = M0 if m == 0 else L
        start = r0
        while start < r1:
            end = min(start + WIN, r1)
            pieces = []
            for p in range(N_POS):
                boff, c0, w = blocks[(p, m)]
                blen = c0 + w
                lo2, hi2 = max(boff, start), min(boff + blen, end)
                if lo2 < hi2:
                    pieces.append((p, lo2, hi2))
            windows.append((m, start, end, pieces))
            start = end
    # interleave mask0/mask1 windows for a smoother engine mix
    w0 = [w for w in windows if w[0] == 0]
    w1 = [w for w in windows if w[0] == 1]
    inter = []
    for i in range(max(len(w0), len(w1))):
        if i < len(w0):
            inter.append(w0[i])
        if i < len(w1):
            inter.append(w1[i])
    windows = inter

    slot_idx = np.full((NC, L), -1, np.int64)
    for c in range(NC):
        for p in range(N_POS):
            for m in range(2):
                for k in range(3):
                    ii = idx[(c, p, m, k)]
                    o = sub_off[(p, m, k)]
                    slot_idx[c, o:o + len(ii)] = ii

    ain_slot = np.full(Lin, -1, np.int64)
    ain_kind = np.zeros(Lin, np.int64)
    ain_slot[0:L] = np.arange(L)
    ain_kind[0:L] = 0
    ain_slot[L:2 * L] = np.arange(L)
    ain_kind[L:2 * L] = 1
    for m in range(2):
        for p in range(N_POS):
            boff, c0, w = blocks[(p, m)]
            a = 2 * L + e_off[(p, m)]
            ain_slot[a:a + w] = np.arange(boff + c0, boff + c0 + w)
            ain_kind[a:a + w] = 2

    out_slot = np.full(Lout, -1, np.int64)
    out_kind = np.zeros(Lout, np.int64)
    out_slot[0:M0] = np.arange(M0)
    out_kind[0:M0] = 0
    out_slot[M0:2 * M0] = np.arange(M0)
    out_kind[M0:2 * M0] = 1
    out_slot[2 * M0:] = np.arange(M0, L)
    out_kind[2 * M0:] = 2

    return dict(L=L, M0=M0, Lin=Lin, Lout=Lout, Etot=Etot, blocks=blocks,
                e_off=e_off, windows=windows, slot_idx=slot_idx,
                ain_slot=ain_slot, ain_kind=ain_kind,
                out_slot=out_slot, out_kind=out_kind)


def _build(plan, reps=1, bodies=1):
    L, M0, Lin, Lout = plan["L"], plan["M0"], plan["Lin"], plan["Lout"]
    blocks, e_off, windows = plan["blocks"], plan["e_off"], plan["windows"]
    WCOLS = N_POS * 4 * H          # per-pos weight blocks
    BF_OFF = WCOLS                 # b_f: row 0, per pos at BF_OFF + p*H

    nc = bacc.Bacc("TRN2", target_bir_lowering=False)
    AIN = nc.dram_tensor("AIN", [H, Lin], F16, kind="ExternalInput")
    W = nc.dram_tensor("W", [H, WCOLS + N_POS * H], F16,
                       kind="ExternalInput")
    OH = nc.dram_tensor("OH", [1, WIN], F16, kind="ExternalInput")
    BIAS = nc.dram_tensor("BIAS", [H, 3], F32, kind="ExternalInput")
    OUT = nc.dram_tensor("OUT", [H, Lout], F16, kind="ExternalOutput")

    with TileContext(nc) as tc:
        with (
            tc.tile_pool(name="const", bufs=1) as cpool,
            tc.tile_pool(name="io", bufs=2) as io,
            tc.tile_pool(name="ot", bufs=2) as ot,
            tc.tile_pool(name="wk", bufs=4) as wk,
            tc.tile_pool(name="ps_u", bufs=2, space="PSUM") as ps_u,
            tc.tile_pool(name="ps_i", bufs=2, space="PSUM") as ps_i,
            tc.tile_pool(name="ps_o", bufs=2, space="PSUM") as ps_o,
            tc.tile_pool(name="ps_t", bufs=2, space="PSUM") as ps_t,
        ):
            w_sb = cpool.tile([H, WCOLS + N_POS * H], F16, tag="w")
            nc.sync.dma_start(out=w_sb[:, :], in_=W[:, :])
            oh_sb = cpool.tile([1, WIN], F16, tag="oh")
            nc.sync.dma_start(out=oh_sb[:, :], in_=OH[:, :])
            bias_sb = cpool.tile([H, 3], F32, tag="bias")
            nc.sync.dma_start(out=bias_sb[:, :], in_=BIAS[:, :])

            def body(_iv=None):
                ain = io.tile([H, Lin], F16, tag="ain")
                nc.sync.dma_start(out=ain[:, :], in_=AIN[:, :])
                out = ot.tile([H, Lout], F16, tag="out")

                # extras adds on the d1/d2 tails of each (pos, mask) block
                for m in range(2):
                    for p in range(N_POS):
                        boff, c0, w = blocks[(p, m)]
                        if w:
                            t0 = boff + c0
                            a = 2 * L + e_off[(p, m)]
                            nc.vector.tensor_add(
                                ain[:, t0:t0 + w], ain[:, t0:t0 + w],
                                ain[:, a:a + w])

                for (m, lo, hi, pieces) in windows:
                    C = hi - lo
                    p_u = ps_u.tile([H, C], F32, tag="u")
                    for (p, plo, phi) in pieces:
                        ap = p_u[:, plo - lo:phi - lo]
                        nc.tensor.matmul(
                            ap, w_sb[:, p * 4 * H:p * 4 * H + H],
                            ain[:, plo:phi], start=True, stop=False)
                        # rank-1 b_f accumulate against a ones row
                        nc.tensor.matmul(
                            ap, w_sb[0:1, BF_OFF + p * H:BF_OFF + (p + 1) * H],
                            oh_sb[0:1, 0:phi - plo], start=False, stop=True)
                    f_sb = wk.tile([H, C], F16, tag="f")
                    nc.scalar.activation(f_sb[:, :], p_u[:, :], Sig)

                    cL = L + lo  # child_c columns
                    if m == 0:
                        cr_sb = wk.tile([H, C], F16, tag="cr")
                        nc.vector.tensor_mul(cr_sb[:, :], f_sb[:, :],
                                             ain[:, cL:cL + C])
                        p_i = ps_i.tile([H, C], F32, tag="i")
                        p_o = ps_o.tile([H, C], F32, tag="o")
                        p_t = ps_t.tile([H, C], F32, tag="t")
                        for (p, plo, phi) in pieces:
                            base = p * 4 * H
                            nc.tensor.matmul(
                                p_i[:, plo - lo:phi - lo],
                                w_sb[:, base + H:base + 2 * H],
                                ain[:, plo:phi], start=True, stop=True)
                            nc.tensor.matmul(
                                p_o[:, plo - lo:phi - lo],
                                w_sb[:, base + 2 * H:base + 3 * H],
                                ain[:, plo:phi], start=True, stop=True)
                            nc.tensor.matmul(
                                p_t[:, plo - lo:phi - lo],
                                w_sb[:, base + 3 * H:base + 4 * H],
                                ain[:, plo:phi], start=True, stop=True)
                        si_sb = wk.tile([H, C], F16, tag="si")
                        nc.scalar.activation(si_sb[:, :], p_i[:, :], Sig,
                                             bias=bias_sb[:, 0:1])
                        tu_sb = wk.tile([H, C], F16, tag="tu")
                        nc.scalar.activation(tu_sb[:, :], p_t[:, :], Tanh,
                                             bias=bias_sb[:, 2:3])
                        nc.vector.tensor_mul(si_sb[:, :], si_sb[:, :],
                                             tu_sb[:, :])
                        c_new = out[:, M0 + lo:M0 + hi]
                        nc.vector.tensor_add(c_new, si_sb[:, :], cr_sb[:, :])
                        so_sb = wk.tile([H, C], F16, tag="so")
                        nc.scalar.activation(so_sb[:, :], p_o[:, :], Sig,
                                             bias=bias_sb[:, 1:2])
                        th_sb = wk.tile([H, C], F16, tag="th")
                        nc.scalar.activation(th_sb[:, :], c_new, Tanh)
                        nc.vector.tensor_mul(out[:, lo:hi], so_sb[:, :],
                                             th_sb[:, :])
                    else:
                        # c_red straight into OUT: col = 2*M0 + (lo - M0)
                        o0 = M0 + lo
                        nc.vector.tensor_mul(out[:, o0:o0 + C], f_sb[:, :],
                                             ain[:, cL:cL + C])

                nc.sync.dma_start(out=OUT[:, :], in_=out[:, :])

            if reps == 1:
                body()
            else:
                with tc.For_i(0, reps, 1) as _i:
                    for _ in range(bodies):
                        body(_i)
    nc.finalize()
    return nc


_BUILD_CACHE = {}


def _prepare(inputs, reps=1, bodies=1):
    global N, H, N_POS, SH
    N, _, H = np.asarray(inputs["child_h"]).shape
    N_POS = np.asarray(inputs["W_f"]).shape[0] // H
    SH = N // NC
    child_h = np.asarray(inputs["child_h"], np.float32).reshape(N, H)
    child_c = np.asarray(inputs["child_c"], np.float32).reshape(N, H)
    e1 = np.asarray(inputs["extra_input_depth_1"], np.float32)
    e2 = np.asarray(inputs["extra_input_depth_2"], np.float32)
    h_prev = np.asarray(inputs["h_prev"], np.float32)
    pos = np.asarray(inputs["pos"]).astype(np.int64)
    depth = np.asarray(inputs["depth"]).astype(np.int64)
    mask = np.asarray(inputs["mask"]).astype(np.int64)
    W_f = np.asarray(inputs["W_f"], np.float32)
    b_f = np.asarray(inputs["b_f"], np.float32)
    W_iou = np.asarray(inputs["W_iou"], np.float32)
    b_iou = np.asarray(inputs["b_iou"], np.float32)

    mask01 = (mask != 0).astype(np.int64)
    plan = _plan(pos, depth, mask01)
    L, M0, Lin, Lout = plan["L"], plan["M0"], plan["Lin"], plan["Lout"]

    key = (tuple(sorted((k, v) for k, v in plan["blocks"].items())),
           Lin, Lout, reps, bodies)
    if key not in _BUILD_CACHE:
        _BUILD_CACHE[key] = _build(plan, reps=reps, bodies=bodies)
    nc = _BUILD_CACHE[key]

    # weights fp16 [H, 10*4*H + 10*H]: per pos [W_f | WiT | WoT | WuT];
    # then b_f on row 0, per pos at WCOLS + p*H (rank-1 bias matmuls)
    Wp = np.zeros((H, N_POS * 4 * H + N_POS * H), np.float16)
    W_f_r = W_f.reshape(N_POS, H, H)
    b_f_r = b_f.reshape(N_POS, H)
    for p in range(N_POS):
        base = p * 4 * H
        Wp[:, base:base + H] = W_f_r[p]
        for j in range(3):
            Wp[:, base + (j + 1) * H:base + (j + 2) * H] = \
                W_iou[j * H:(j + 1) * H, p * H:(p + 1) * H].T
    Wp[0, N_POS * 4 * H:] = b_f_r.reshape(-1)

    OHm = np.ones((1, WIN), np.float16)

    bias = np.empty((H, 3), np.float32)
    bias[:, 0] = b_iou[0, 0:H]
    bias[:, 1] = b_iou[0, H:2 * H]
    bias[:, 2] = b_iou[0, 2 * H:3 * H]

    e_src = np.where((depth == 1)[:, None], e1, e2).astype(np.float32)
    srcs = (child_h, child_c, e_src)

    slot_idx = plan["slot_idx"]
    ain_slot, ain_kind = plan["ain_slot"], plan["ain_kind"]
    out_slot, out_kind = plan["out_slot"], plan["out_kind"]

    in_maps = []
    for c in range(NC):
        node = np.where(ain_slot >= 0, slot_idx[c][ain_slot], -1)
        AINm = np.zeros((H, Lin), np.float16)
        for kind in range(3):
            mm = (ain_kind == kind) & (node >= 0)
            AINm[:, mm] = srcs[kind][node[mm]].T.astype(np.float16)
        in_maps.append({"AIN": AINm, "W": Wp, "OH": OHm, "BIAS": bias})

    mask_on = mask != 0

    def assemble(results):
        h = np.empty((N, H), np.float32)
        cc = np.empty((N, H), np.float32)
        for c in range(NC):
            node = slot_idx[c][out_slot]
            O = results[c]["OUT"]
            mh = (out_kind == 0) & (node >= 0)
            h[node[mh]] = O[:, mh].T.astype(np.float32)
            mc = (out_kind != 0) & (node >= 0)
            cc[node[mc]] = O[:, mc].T.astype(np.float32)
        h[mask_on] = h_prev[mask_on]
        return h, cc

    return nc, in_maps, assemble


def kernel(**inputs):
    nc, in_maps, assemble = _prepare(inputs)
    try:
        res = run_bass_kernel_spmd(nc, in_maps, list(range(NC)))
    except Exception:
        # first execution of a freshly compiled NEFF occasionally kills the
        # worker (transient); one retry has always succeeded
        res = run_bass_kernel_spmd(nc, in_maps, list(range(NC)))
    LAST["results"] = res
    LAST["nc"] = nc
    return assemble(res.results)
